# revision 15
# baseline (speedup 1.0000x reference)
"""Trainium2 Bass kernel for a sparse-attention (sliding-window) transformer block.

v2: bf16 storage end-to-end (PSUM accumulation stays f32), band masks applied
via PE matmul-accumulate of constant triangle bias matrices, 128-query attention
blocks x 3 key tiles, deferred softmax renorm folded into the gate multiply,
Silu activation, resident wo with j-outer output loop, PE warmup matmuls.

Sharding: token-parallel across 8 cores (B=2 x 4 chunks of 512 tokens), each
core gets a 256-token halo and recomputes K/V on it -> no collectives.
Out-of-range halo tokens are zeros; their keys are suppressed exactly by
zeroing the ones-column of V (denominator sees 0, numerator sees v=0).
"""

import os
import sys

import numpy as np

if "/opt/trn_rl_repo" not in sys.path:
    sys.path.insert(0, "/opt/trn_rl_repo")

# ---- problem constants (hardcoded; kernel.py must be self-contained) ----
D = 1024          # d_model
NH = 16           # heads
DH = 64           # head dim
DFF = 4096        # ffn hidden
WIN = 256         # sliding window
B, T = 2, 2048
EPS = 1e-6

NCORES = 8
CHUNK = 512       # own tokens per core
HALO = 256        # preceding-token halo
LT = CHUNK + HALO  # 768 local tokens (halo first)
P = 128
ND = D // P       # 8
NF = DFF // P     # 32
NT = LT // P      # 6 local token tiles

EXP_SHIFT = -20.0  # constant shift inside exp; softmax-invariant
SCALE = DH ** -0.5
MASKVAL = -1e6     # additive pre-scale mask bias (exp underflows to 0)

_CACHE = {}


# --------------------------------------------------------------------------
# program builder
# --------------------------------------------------------------------------

def build_program():
    import concourse.bacc as bacc
    import concourse.tile as tile
    from concourse import mybir

    f32 = mybir.dt.float32
    bf16 = mybir.dt.bfloat16

    nc = bacc.Bacc("TRN2", target_bir_lowering=False, debug=False,
                   num_devices=NCORES)

    io = {}
    io["xT"] = nc.dram_tensor("xT", [P, ND, LT], bf16, kind="ExternalInput").ap()
    # per-output-tile pre-tiled weights: [tile, p, a(in-tile), o]
    io["wqS"] = nc.dram_tensor("wqS", [ND, P, ND, P], bf16, kind="ExternalInput").ap()
    io["wkS"] = nc.dram_tensor("wkS", [ND, P, ND, P], bf16, kind="ExternalInput").ap()
    io["wvS"] = nc.dram_tensor("wvS", [4, P, ND, 256], bf16, kind="ExternalInput").ap()
    io["wgateS"] = nc.dram_tensor("wgateS", [ND, P, ND, P], bf16, kind="ExternalInput").ap()
    io["woutS"] = nc.dram_tensor("woutS", [P, ND, ND, P], bf16, kind="ExternalInput").ap()
    io["wgS"] = nc.dram_tensor("wgS", [16, P, ND, 256], bf16, kind="ExternalInput").ap()
    io["wuS"] = nc.dram_tensor("wuS", [16, P, ND, 256], bf16, kind="ExternalInput").ap()
    io["woS"] = nc.dram_tensor("woS", [P, NF, ND, P], bf16, kind="ExternalInput").ap()
    # triangle band masks (additive, pre-scale): [p(c), {j0,j2}, r]
    io["maskM"] = nc.dram_tensor("maskM", [P, 2, P], bf16, kind="ExternalInput").ap()
    io["ident"] = nc.dram_tensor("ident", [P, P], bf16, kind="ExternalInput").ap()
    io["sel16"] = nc.dram_tensor("sel16", [NH, ND, P], bf16, kind="ExternalInput").ap()
    io["dstg"] = nc.dram_tensor("dstg", [NH, CHUNK], mybir.dt.float32, kind="Internal").ap()
    # validity column for V (1.0 valid token, 0.0 zero-pad halo), bcast over heads
    io["vones"] = nc.dram_tensor("vones", [P, NT, NH], f32, kind="ExternalInput").ap()
    io["outT"] = nc.dram_tensor("outT", [P, ND, CHUNK], f32, kind="ExternalOutput").ap()

    with tile.TileContext(nc) as tc:
        nloop = int(os.environ.get("BASS_BODY_LOOP", "1"))
        if nloop > 1:
            import contextlib
            loop_ctx = tc.For_i(0, nloop)
        else:
            import contextlib
            loop_ctx = contextlib.nullcontext()
        with loop_ctx:
            for _ in range(int(os.environ.get("BASS_BODY_REPEAT", "1"))):
                try:
                    _emit(tc, io)
                except Exception as e:
                    if type(e).__name__ != "_Done":
                        raise

    nc.compile()
    return nc


def _emit(tc, io):
    from contextlib import ExitStack

    from concourse import mybir

    nc = tc.nc
    f32 = mybir.dt.float32
    bf16 = mybir.dt.bfloat16
    AF = mybir.ActivationFunctionType
    LIMIT = int(os.environ.get("BASS_PHASE_LIMIT", "9"))

    class _Done(Exception):
        pass

    def dump_and_stop(tc_, limit):
        if LIMIT > limit:
            return
        with tc_.tile_pool(name="dump", bufs=1) as dp:
            t = dp.tile([P, CHUNK], f32)
            nc.vector.memset(t, 0.0)
            for j in range(ND):
                nc.sync.dma_start(out=io["outT"][:, j, :], in_=t[:])
        raise _Done

    with ExitStack() as ctx:
        glob = ctx.enter_context(tc.tile_pool(name="glob", bufs=1))

        # PE warmup fodder first: warmups gate only on this memset
        wscr = glob.tile([P, 512], bf16)
        nc.vector.memset(wscr, 0.0)

        ones = glob.tile([P, 1], bf16)
        nc.vector.memset(ones, 1.0)
        ones1 = glob.tile([1, P], bf16)
        nc.vector.memset(ones1, 1.0)
        epsb = glob.tile([P, 1], f32)
        nc.vector.memset(epsb, EPS)
        shiftb = glob.tile([P, 1], f32)
        nc.vector.memset(shiftb, EXP_SHIFT)

        maskM = glob.tile([P, 2, P], bf16)
        nc.sync.dma_start(out=maskM[:], in_=io["maskM"])
        ident = glob.tile([P, P], bf16)
        nc.sync.dma_start(out=ident[:], in_=io["ident"])
        sel16 = glob.tile([NH, ND, P], bf16)
        nc.sync.dma_start(out=sel16[:], in_=io["sel16"])
        vones = glob.tile([P, NT, NH], f32)
        nc.sync.dma_start(out=vones[:], in_=io["vones"])

        xT = glob.tile([P, ND, LT], bf16)
        nc.sync.dma_start(out=xT[:, 0:4, :], in_=io["xT"][:, 0:4, :])
        nc.sync.dma_start(out=xT[:, 4:ND, :], in_=io["xT"][:, 4:ND, :])

        # h2 outlives the attention pools (opened first for stack order)
        h2T = ctx.enter_context(tc.tile_pool(name="h2T", bufs=1)).tile(
            [P, ND, CHUNK], bf16, name="h2T")
        wt4S = ctx.enter_context(tc.tile_pool(name="wt4S", bufs=1)).tile(
            [P, ND, ND, P], bf16, name="wt4S")

        with ExitStack() as actx:
            att = actx
            h1T = att.enter_context(tc.tile_pool(name="h1T", bufs=1)).tile(
                [P, ND, LT], bf16, name="h1T")
            qT = att.enter_context(tc.tile_pool(name="qT", bufs=1)).tile(
                [P, ND, CHUNK], bf16, name="qT")
            kT = att.enter_context(tc.tile_pool(name="kT", bufs=1)).tile(
                [P, ND, LT], bf16, name="kT")
            vaug = att.enter_context(tc.tile_pool(name="vaug", bufs=1)).tile(
                [P, NT, NH, DH + 1], bf16, name="vaug")
            gateT = att.enter_context(tc.tile_pool(name="gateT", bufs=1)).tile(
                [P, ND, CHUNK], bf16, name="gateT")
            attT = att.enter_context(tc.tile_pool(name="attT", bufs=1)).tile(
                [P, ND, CHUNK], bf16, name="attT")
            denom1 = att.enter_context(tc.tile_pool(name="denom1", bufs=1)).tile(
                [1, NH, CHUNK], f32, name="denom1")
            denT = att.enter_context(tc.tile_pool(name="denT", bufs=1)).tile(
                [NH, CHUNK], f32, name="denT")
            rrec16 = att.enter_context(tc.tile_pool(name="rrec16", bufs=1)).tile(
                [NH, CHUNK], bf16, name="rrec16")

            # ones(+validity) column of V
            nc.vector.tensor_copy(
                vaug[:, :, :, DH:DH + 1].rearrange("p a h o -> p (a h o)"),
                vones[:].rearrange("p a h -> p (a h)"))

            # ---------------- phase 1: rmsnorm over all LT tokens ----------
            with ExitStack() as pctx:
                sqp = pctx.enter_context(tc.tile_pool(name="sqp", bufs=3))
                msp = pctx.enter_context(tc.tile_pool(name="msp", bufs=2, space="PSUM"))
                rowp = pctx.enter_context(tc.tile_pool(name="rowp", bufs=2))
                wup = pctx.enter_context(tc.tile_pool(name="wup", bufs=2, space="PSUM"))

                def warm(n):
                    for _ in range(n):
                        w = wup.tile([P, 512], f32, name="wps", tag="wps")
                        nc.tensor.matmul(w[:], wscr[:, 0:P], wscr[:],
                                         start=True, stop=True)

                warm(8)
                rbcp = pctx.enter_context(tc.tile_pool(name="rbcp", bufs=2,
                                                       space="PSUM"))
                rrow = rowp.tile([1, LT], bf16, name="rrow")
                for g in range(2):  # token groups of 384
                    sl = slice(g * 384, (g + 1) * 384)
                    ms = msp.tile([1, 384], f32, name="ms")
                    for p in range(ND):
                        sq = sqp.tile([P, 384], bf16, name="sq")
                        if p % 2 == 0:
                            nc.vector.tensor_mul(sq[:], xT[:, p, sl],
                                                 xT[:, p, sl])
                        else:
                            nc.scalar.activation(sq[:], xT[:, p, sl], AF.Square)
                        nc.tensor.matmul(ms[:], ones[:], sq[:],
                                         start=(p == 0), stop=(p == ND - 1))
                        if p % 2 == 1:
                            warm(1)
                    sr = rowp.tile([1, 384], f32, name="sr")
                    nc.scalar.activation(sr[:], ms[:], AF.Sqrt,
                                         bias=epsb[0:1, :], scale=1.0 / D)
                    with nc.allow_low_precision(reason="rms scale in bf16"):
                        nc.vector.reciprocal(rrow[:, sl], sr[:])
                    rbc = rbcp.tile([P, 384], f32, name="rbc")
                    nc.tensor.matmul(rbc[:], ones1[:], rrow[:, sl],
                                     start=True, stop=True)
                    warm(2)
                    for p in range(ND):
                        nc.vector.tensor_mul(h1T[:, p, sl], xT[:, p, sl],
                                             rbc[:])

            dump_and_stop(tc, 1)
            # ------ phases 2+3 merged: projections interleaved with ------
            # ------ attention so ACT/DVE softmax work hides under PE ------
            with ExitStack() as pctx:
                wsp = pctx.enter_context(tc.tile_pool(name="wsp", bufs=3))
                wvp = pctx.enter_context(tc.tile_pool(name="wvp", bufs=1))
                wkp = pctx.enter_context(tc.tile_pool(name="wkp", bufs=1))
                pj = pctx.enter_context(tc.tile_pool(name="pj", bufs=2, space="PSUM"))
                stp = pctx.enter_context(tc.tile_pool(name="stp", bufs=2, space="PSUM"))
                avp = pctx.enter_context(tc.tile_pool(name="avp", bufs=2, space="PSUM"))
                ptp = pctx.enter_context(tc.tile_pool(name="ptp", bufs=3))
                wvt = []
                wkt = []

                def kpass(g):
                    sl = slice(g * 384, (g + 1) * 384)
                    for po in range(ND):
                        if g == 0:
                            wt = wkp.tile([P, ND, P], bf16, name="wk",
                                          tag=f"wk{po}")
                            wkt.append(wt)
                            nc.sync.dma_start(out=wt[:], in_=io["wkS"][po])
                        kg1(po, g)

                def kg1(po, g=1):
                    sl = slice(g * 384, (g + 1) * 384)
                    ps = pj.tile([P, 384], f32, name="psk", tag="pj")
                    for pi in range(ND):
                        nc.tensor.matmul(ps[:], wkt[po][:, pi, :],
                                         h1T[:, pi, sl],
                                         start=(pi == 0), stop=(pi == ND - 1))
                    nc.scalar.copy(kT[:, po, sl], ps[:])

                def vgroup(g, ng):
                    if g == 0 and len(wvt) <= ng:
                        wv = wvp.tile([P, ND, 256], bf16, name="wv",
                                      tag=f"wv{ng}")
                        wvt.append(wv)
                        nc.sync.dma_start(out=wv[:], in_=io["wvS"][ng])
                    wv = wvt[ng]
                    for tt in range(3 * g, 3 * g + 3):
                        ps = pj.tile([P, 256], f32, name="psv", tag="pj")
                        for pi in range(ND):
                            nc.tensor.matmul(
                                ps[:], h1T[:, pi, tt * P:(tt + 1) * P],
                                wv[:, pi, :],
                                start=(pi == 0), stop=(pi == ND - 1))
                        nc.vector.tensor_copy(
                            vaug[:, tt, 4 * ng:4 * (ng + 1), 0:DH],
                            ps[:].rearrange("p (h d) -> p h d", d=DH))

                def qproj(po):
                    wt = wsp.tile([P, ND, P], bf16, name="wt", tag="wt")
                    nc.sync.dma_start(out=wt[:], in_=io["wqS"][po])
                    ps = pj.tile([P, CHUNK], f32, name="ps", tag="pj")
                    for pi in range(ND):
                        nc.tensor.matmul(ps[:], wt[:, pi, :],
                                         h1T[:, pi, HALO:LT],
                                         start=(pi == 0), stop=(pi == ND - 1))
                    nc.vector.tensor_copy(qT[:, po, :], ps[:])

                att_pend = []

                def att_fire(h, sb):
                    poh, off = h // 2, (h % 2) * DH
                    st = stp.tile([P, 2, 3, P], f32, name="st")
                    for bi in range(2):
                        b = 2 * sb + bi
                        qsl = slice(b * P, (b + 1) * P)
                        for j in range(3):
                            nc.tensor.matmul(
                                st[:, bi, j, :],
                                kT[off:off + DH, poh, (b + j) * P:(b + j + 1) * P],
                                qT[off:off + DH, poh, qsl],
                                start=True, stop=(j == 1))
                            if j != 1:
                                nc.tensor.matmul(st[:, bi, j, :], ident[:],
                                                 maskM[:, j // 2, :],
                                                 start=False, stop=True)
                    pt = ptp.tile([P, 2, 3, P], bf16, name="pt")
                    nc.scalar.activation(pt[:], st[:], AF.Exp,
                                         bias=shiftb[:], scale=SCALE)
                    att_pend.append((h, sb, pt))

                def att_tail():
                    h, sb, pt = att_pend.pop(0)
                    poh, off = h // 2, (h % 2) * DH
                    av = avp.tile([DH + 1, 2, P], f32, name="av")
                    for bi in range(2):
                        b = 2 * sb + bi
                        for j in range(3):
                            nc.tensor.matmul(av[:, bi, :],
                                             vaug[:, b + j, h, :],
                                             pt[:, bi, j, :],
                                             start=(j == 0), stop=(j == 2))
                    qs = slice(sb * 256, (sb + 1) * 256)
                    nc.scalar.copy(
                        attT[off:off + DH, poh, qs],
                        av[0:DH, :, :].rearrange("p a b -> p (a b)"))
                    nc.vector.tensor_copy(
                        denom1[0:1, h, qs],
                        av[DH:DH + 1, :, :].rearrange("p a b -> p (a b)"))

                def att_head(h):
                    # 2-stage software pipeline: fire QK/exp for (h, sb), and
                    # run the AV+copies of the PREVIOUS super-block so the
                    # in-order PE queue never waits on the exp
                    for sb in range(2):
                        att_fire(h, sb)
                        if len(att_pend) > 1:
                            att_tail()

                def gproj(po):
                    # gate^T = sigmoid(z) = 1/(1+exp(-z)) (Exp table loaded)
                    wt = wsp.tile([P, ND, P], bf16, name="wt", tag="wt")
                    nc.sync.dma_start(out=wt[:], in_=io["wgateS"][po])
                    ps = pj.tile([P, CHUNK], f32, name="ps", tag="pj")
                    for pi in range(ND):
                        nc.tensor.matmul(ps[:], wt[:, pi, :],
                                         h1T[:, pi, HALO:LT],
                                         start=(pi == 0), stop=(pi == ND - 1))
                    nc.scalar.activation(gateT[:, po, :], ps[:], AF.Exp,
                                         scale=-1.0)
                    nc.vector.tensor_scalar_add(gateT[:, po, :],
                                                gateT[:, po, :], 1.0)
                    with nc.allow_low_precision(reason="gate sigmoid bf16"):
                        nc.vector.reciprocal(gateT[:, po, :], gateT[:, po, :])

                kpass(0)
                vgroup(0, 0)
                vgroup(0, 1)
                vgroup(0, 2)
                vgroup(0, 3)
                qproj(0)
                qproj(1)
                gproj(0)
                for po in range(ND):
                    kg1(po)
                    if po % 2 == 0:
                        vgroup(1, po // 2)
                    if po < ND - 2:
                        qproj(po + 2)
                    if po < ND - 1:
                        gproj(po + 1)
                    att_head(2 * po)
                    att_head(2 * po + 1)
                while att_pend:
                    att_tail()

                dump_and_stop(tc, 2)
                # prefetch the out-proj weights; consumed in phase 4
                nc.sync.dma_start(out=wt4S[:], in_=io["woutS"])

            dump_and_stop(tc, 3)

            # renorm + gate fold, denominators transposed via a DRAM
            # round-trip (SBUF->SBUF partition-expanding DMA corrupts on HW)
            with ExitStack() as pctx:
                rcp = pctx.enter_context(tc.tile_pool(name="rcp", bufs=2, space="PSUM"))
                nc.sync.dma_start(
                    out=io["dstg"].rearrange("h t -> (h t)"),
                    in_=denom1[:].rearrange("o h t -> o (h t)"))
                nc.sync.dma_start(out=denT[:], in_=io["dstg"])
                with nc.allow_low_precision(reason="softmax denom recip bf16"):
                    nc.vector.reciprocal(rrec16[:], denT[:])
                for po in range(ND):
                    rb = rcp.tile([P, CHUNK], f32, name="rb")
                    nc.tensor.matmul(rb[:], sel16[:, po, :], rrec16[:],
                                     start=True, stop=True)
                    nc.vector.tensor_mul(gateT[:, po, :], gateT[:, po, :],
                                         rb[:])
                    eng = nc.vector if po >= ND - 2 else nc.gpsimd
                    eng.tensor_mul(attT[:, po, :], attT[:, po, :],
                                   gateT[:, po, :])

            dump_and_stop(tc, 3)
            # preload the Sqrt act table while ACT is idle (rmsnorm2's real
            # Sqrt would otherwise pay the table switch on the critical path)
            scrp_ = tc.tile_pool(name="scr1", bufs=1)
            with scrp_ as scrp:
                scr1 = scrp.tile([1, 1], f32, name="scr1")
                nc.scalar.activation(scr1[:], epsb[0:1, :], AF.Sqrt)

            # ------------- phase 4: out-proj, residual, rmsnorm2, h2 ------
            # (still inside the attention ExitStack: consumes attT, then the
            # attention tensors are released before the FFN pools open)
            with ExitStack() as pctx:
                wsp = pctx.enter_context(tc.tile_pool(name="wsp4", bufs=3))
                pj = pctx.enter_context(tc.tile_pool(name="pj4", bufs=3, space="PSUM"))
                ms2p = pctx.enter_context(tc.tile_pool(name="ms2p", bufs=1, space="PSUM"))
                sqp = pctx.enter_context(tc.tile_pool(name="sqp5", bufs=3))
                rowp = pctx.enter_context(tc.tile_pool(name="rowp5", bufs=2))

                ms2 = ms2p.tile([1, CHUNK], f32, name="ms2")
                for pjx in range(ND):
                    ps = pj.tile([P, CHUNK], f32, name="ps4")
                    for po in range(ND):
                        nc.tensor.matmul(ps[:], wt4S[:, pjx, po, :],
                                         attT[:, po, :],
                                         start=(po == 0), stop=(po == ND - 1))
                    # x2 = x + attn_out, in place into xT's own-token region
                    nc.vector.tensor_add(xT[:, pjx, HALO:LT], ps[:],
                                         xT[:, pjx, HALO:LT])
                    sq = sqp.tile([P, CHUNK], bf16, name="sq5")
                    nc.vector.tensor_mul(sq[:], xT[:, pjx, HALO:LT],
                                         xT[:, pjx, HALO:LT])
                    nc.tensor.matmul(ms2[:], ones[:], sq[:],
                                     start=(pjx == 0), stop=(pjx == ND - 1))

                sr = rowp.tile([1, CHUNK], f32, name="sr5")
                nc.scalar.activation(sr[:], ms2[:], AF.Sqrt, bias=epsb[0:1, :],
                                     scale=1.0 / D)
                rrow2 = rowp.tile([1, CHUNK], bf16, name="rrow5")
                with nc.allow_low_precision(reason="rms scale in bf16"):
                    nc.vector.reciprocal(rrow2[:], sr[:])
                rbc2p = pctx.enter_context(tc.tile_pool(name="rbc2p", bufs=1,
                                                        space="PSUM"))
                rbc2 = rbc2p.tile([P, CHUNK], f32, name="rbc5")
                nc.tensor.matmul(rbc2[:], ones1[:], rrow2[:],
                                 start=True, stop=True)
                for p in range(ND):
                    nc.vector.tensor_mul(h2T[:, p, :], xT[:, p, HALO:LT],
                                         rbc2[:])

            dump_and_stop(tc, 4)

        # attention tensors released here
        # ---------------- phase 5: swiglu ffn -----------------------------
        with ExitStack() as fctx:
            prodp = fctx.enter_context(tc.tile_pool(name="prod", bufs=1))
            prod = prodp.tile([P, NF, CHUNK], bf16, name="prod")
            wopool = fctx.enter_context(tc.tile_pool(name="wopool", bufs=1))
            woS = wopool.tile([P, NF, ND, P], bf16, name="woS")

            with ExitStack() as pctx:
                wgp = pctx.enter_context(tc.tile_pool(name="wgp", bufs=2))
                wgu = pctx.enter_context(tc.tile_pool(name="wgu", bufs=2))
                gsp = pctx.enter_context(tc.tile_pool(name="gsp", bufs=3))
                pg = pctx.enter_context(tc.tile_pool(name="pg", bufs=2, space="PSUM"))
                pu = pctx.enter_context(tc.tile_pool(name="pu", bufs=2, space="PSUM"))

                for fb in range(16):  # f-blocks of 256 (2 f-tiles each)
                    if 2 <= fb < 10:  # stream wo behind the first wg/wu tiles
                        c = fb - 2
                        nc.sync.dma_start(out=woS[:, 4 * c:4 * (c + 1)],
                                          in_=io["woS"][:, 4 * c:4 * (c + 1)])
                    wg = wgp.tile([P, ND, 256], bf16, name="wg")
                    nc.sync.dma_start(out=wg[:], in_=io["wgS"][fb])
                    wu = wgu.tile([P, ND, 256], bf16, name="wu")
                    nc.sync.dma_start(out=wu[:], in_=io["wuS"][fb])
                    for f4 in range(2):
                        fo = fb * 2 + f4
                        gps = pg.tile([P, CHUNK], f32, name="gps")
                        for pi in range(ND):
                            nc.tensor.matmul(gps[:],
                                             wg[:, pi, f4 * P:(f4 + 1) * P],
                                             h2T[:, pi, :],
                                             start=(pi == 0), stop=(pi == ND - 1))
                        sg = gsp.tile([P, CHUNK], bf16, name="sg", tag="sg")
                        if os.environ.get("BASS_SIM_SILU"):
                            # CoreSim has no Silu: sigmoid + explicit multiply
                            nc.scalar.activation(sg[:], gps[:], AF.Sigmoid)
                            nc.vector.tensor_mul(sg[:], sg[:], gps[:])
                        else:
                            nc.scalar.activation(sg[:], gps[:], AF.Silu)
                        ups = pu.tile([P, CHUNK], f32, name="ups")
                        for pi in range(ND):
                            nc.tensor.matmul(ups[:],
                                             wu[:, pi, f4 * P:(f4 + 1) * P],
                                             h2T[:, pi, :],
                                             start=(pi == 0), stop=(pi == ND - 1))
                        nc.vector.tensor_mul(prod[:, fo, :], sg[:], ups[:])

            with ExitStack() as pctx:
                pz = pctx.enter_context(tc.tile_pool(name="pz", bufs=2, space="PSUM"))
                outp = pctx.enter_context(tc.tile_pool(name="outp", bufs=3))

                for j in range(ND):
                    z2 = pz.tile([P, CHUNK], f32, name="z2")
                    for fo in range(NF):
                        nc.tensor.matmul(z2[:], woS[:, fo, j, :],
                                         prod[:, fo, :],
                                         start=(fo == 0), stop=(fo == NF - 1))
                    ot = outp.tile([P, CHUNK], f32, name="ot")
                    nc.vector.tensor_add(ot[:], z2[:], xT[:, j, HALO:LT])
                    nc.sync.dma_start(out=io["outT"][:, j, :], in_=ot[:])


# --------------------------------------------------------------------------
# host-side sharding / unsharding
# --------------------------------------------------------------------------

import ml_dtypes

_BF16 = np.dtype(ml_dtypes.bfloat16)


def make_shared(ln1_w, qkv_w, gate_w, out_w, ln2_w, wg, wu, wo):
    tot = NH * DH
    wq_e = (qkv_w[0 * tot:1 * tot] * ln1_w[None, :]).T  # [D(in), D(out)]
    wk_e = (qkv_w[1 * tot:2 * tot] * ln1_w[None, :]).T
    wv_e = (qkv_w[2 * tot:3 * tot] * ln1_w[None, :]).T
    wgate_e = (gate_w * ln1_w[None, :]).T
    wout_e = out_w.T                                    # [tot, D]
    wg_e = (wg * ln2_w[None, :]).T                      # [D, DFF]
    wu_e = (wu * ln2_w[None, :]).T
    wo_e = wo.T                                         # [DFF, D]

    def tile_pio(w, ogrp):  # [D_in, N_out] -> [N_out/ogrp, P, D_in/P, ogrp]
        di, no = w.shape
        t = np.asarray(w, np.float32).reshape(di // P, P, no // ogrp, ogrp)
        return np.ascontiguousarray(t.transpose(2, 1, 0, 3)).astype(_BF16)

    # wo: [DFF, D] -> [P(f-within-tile), NF, ND, 128]
    wo_t = np.asarray(wo_e, np.float32).reshape(NF, P, ND, P)
    wo_t = np.ascontiguousarray(wo_t.transpose(1, 0, 2, 3)).astype(_BF16)

    # triangle band masks, additive pre-scale: tile j0 disallows c<=r,
    # tile j2 disallows c>r  (c = key coord in partition, r = query coord)
    c = np.arange(P)[:, None]
    r = np.arange(P)[None, :]
    m = np.zeros((P, 2, P), np.float32)
    m[:, 0, :] = np.where(c <= r, MASKVAL, 0.0)
    m[:, 1, :] = np.where(c > r, MASKVAL, 0.0)

    sel = np.zeros((NH, ND, P), np.float32)
    for po in range(ND):
        sel[2 * po, po, 0:DH] = 1.0
        sel[2 * po + 1, po, DH:P] = 1.0
    sel = sel.astype(_BF16)

    # head-selector for the softmax-denominator broadcast matmul
    sel = np.zeros((NH, ND, P), np.float32)
    for po in range(ND):
        sel[2 * po, po, 0:DH] = 1.0
        sel[2 * po + 1, po, DH:P] = 1.0
    sel = sel.astype(_BF16)

    return {
        "wqS": tile_pio(wq_e, P),
        "wkS": tile_pio(wk_e, P),
        "wvS": tile_pio(wv_e, 256),
        "wgateS": tile_pio(wgate_e, P),
        "woutS": np.ascontiguousarray(
            tile_pio(wout_e, P).transpose(1, 0, 2, 3)),
        "wgS": tile_pio(wg_e, 256),
        "wuS": tile_pio(wu_e, 256),
        "woS": wo_t,
        "maskM": m.astype(_BF16),
        "ident": np.eye(P, dtype=np.float32).astype(_BF16),
        "sel16": sel,
    }


def make_in_maps(x, ln1_w, qkv_w, gate_w, out_w, ln2_w, wg, wu, wo):
    shared = make_shared(ln1_w, qkv_w, gate_w, out_w, ln2_w, wg, wu, wo)

    in_maps = []
    for cidx in range(NCORES):
        b, ck = divmod(cidx, T // CHUNK)
        cs = ck * CHUNK
        xw = np.zeros((LT, D), np.float32)
        lo = cs - HALO
        xw[max(0, -lo):] = x[b, max(lo, 0):cs + CHUNK]
        m = dict(shared)
        # xT pre-tiled [P, ND, LT]
        m["xT"] = np.ascontiguousarray(
            xw.T.reshape(ND, P, LT).transpose(1, 0, 2)).astype(_BF16)
        # validity of each local token (zeros-halo tokens excluded from softmax)
        pos = lo + np.arange(LT)
        valid = (pos >= 0).astype(np.float32)
        m["vones"] = np.ascontiguousarray(
            np.repeat(valid.reshape(NT, P).T[:, :, None], NH, axis=2),
            np.float32)
        in_maps.append(m)
    return in_maps


def gather_output(results):
    out = np.empty((B, T, D), np.float32)
    for cidx in range(NCORES):
        b, ck = divmod(cidx, T // CHUNK)
        o = results[cidx]["outT"]  # [P, ND, CHUNK]
        out[b, ck * CHUNK:(ck + 1) * CHUNK] = \
            o.transpose(2, 1, 0).reshape(CHUNK, D)
    return out


def kernel(**inputs):
    from concourse.bass_utils import run_bass_kernel_spmd

    if "nc" not in _CACHE:
        _CACHE["nc"] = build_program()
    nc = _CACHE["nc"]

    in_maps = make_in_maps(**inputs)
    res = run_bass_kernel_spmd(nc, in_maps, core_ids=list(range(NCORES)))
    return gather_output(res.results)


if __name__ == "__main__":
    rng = np.random.default_rng(0)
    ins = {
        "x": rng.standard_normal((B, T, D), dtype=np.float32),
        "ln1_w": np.ones(D, np.float32),
        "qkv_w": rng.standard_normal((3 * NH * DH, D), dtype=np.float32) * 0.02,
        "gate_w": rng.standard_normal((NH * DH, D), dtype=np.float32) * 0.04,
        "out_w": rng.standard_normal((D, NH * DH), dtype=np.float32) * 0.04,
        "ln2_w": np.ones(D, np.float32),
        "wg": rng.standard_normal((DFF, D), dtype=np.float32) * 0.02,
        "wu": rng.standard_normal((DFF, D), dtype=np.float32) * 0.02,
        "wo": rng.standard_normal((D, DFF), dtype=np.float32) * 0.02,
    }
    out = kernel(**ins)
    print("out", out.shape, out.dtype, float(np.abs(out).mean()))
